# revision 11
# baseline (speedup 1.0000x reference)
"""Trainium2 Bass kernel for nn_EvolvableSNN (T=512, B=8, N=4096, LIF SNN).

Strategy
--------
The LIF dynamics with these parameters are sub-threshold: the membrane
potential equilibrium is ~tau_mem*tau_syn*cur ~= 1e-4 * cur, four orders of
magnitude below threshold=1.0, so no neuron ever spikes and the recurrent
feedback term is identically zero.  With zero feedback the scan is a LINEAR
time-invariant filter of the feedforward drive:

    ff    = input[:, :, :512] @ W_in                      # [T, B, N]
    mem_t = DT^2 * sum_{s<=t} g(t-s) * ff_s               # per (b, n)
    g(d)  = (b^(d+1) - a^(d+1)) / (b - a),  a = 1-DT/tau_syn, b = 1-DT/tau_mem
    spikes_t = (mem_t >= threshold)

so mem = GT.T @_time (x @ W_in) -- two chained dense matmuls, fully parallel
across (batch, neuron).  Validity is guarded by a rigorous norm bound
computed on the host (see kernel()); if the bound does not clear the
threshold by a wide margin -- or the device reports any spike -- we fall
back to an exact sequential numpy port of the reference.  The first spike
of the no-feedback system coincides with the first spike of the true
system, so "no spikes under linearization" exactly implies correctness.

Numerics: both matmul stages run as fp8-e4m3 DoubleRow (2x PE throughput)
with power-of-two scales; fp32 PSUM accumulation.  The threshold is folded
into W on the host (W_eff = W_in / th per column), so the device compare is
always against the single runtime scalar sxx*sgt*sw.  sgt is chosen as
sx/sxx so the stage-1 PSUM->SBUF copy is a PURE CAST (no scale multiply).

Sharding: 4 batch-groups x 2 neuron-column-groups over 8 cores (2 batches
and 2048 columns per core).  The column split halves the per-core weight
DMA (1 MB) -- per-queue DMA bandwidth, not PE time, is what gates an early
stage-2 start.  No collectives.

Schedule lessons baked in (from perfetto traces):
 * The PE HAM clock gate releases (1.2 -> 2.4 GHz) only after ~3.4us of
   GAP-FREE PE activity, and micro-stalls reset the window -- so junk
   warm-up matmuls run during the input DMA and every stage-2 dependency
   (w chunks, xg casts) must be strictly ahead of the PE.
 * Closing a scoped PSUM pool mid-kernel inserts an all-engine barrier --
   one shared pool holds stage-1 and stage-2 tiles.
 * SWDGE (gpsimd) DMA has ~2us fixed cost per transfer; everything
   bandwidth-critical goes on the two HWDGE rings (sync / scalar).
"""

import math

import numpy as np
import ml_dtypes

import concourse.bass as bass
import concourse.mybir as mybir
import concourse.tile as tile
from concourse import bacc, bass_utils

# Problem constants (hardcoded per harness contract).
T, B, N = 512, 8, 4096
IN = 512          # INPUT_SIZE
DT = 0.001
P = 128           # SBUF partitions
NCORES = 8

NBG, NNG = 4, 2   # core grid: batch-groups x neuron-column-groups
NB_LOC = B // NBG          # batches per core (2)
NW = N // NNG              # neuron columns per core (2048)
NCH = NW // 512            # 512-wide w chunks per core (4)
KI = IN // P               # contraction tiles over input dim (4)
KP = KI // 2               # DoubleRow contraction pair-tiles (2)
KT = T // P                # tiles over time dim (4)
N_WARM = 6                 # PE warm-up matmuls during the input DMA
F32 = mybir.dt.float32
FP8 = mybir.dt.float8e4
NPFP8 = ml_dtypes.float8_e4m3

MARGIN = 0.1      # abs margin to threshold 1.0 for the fast path

_compiled = {}    # cached compiled Bass module
LAST_RES = None   # last device results (for external profiling)


def _filter_taps(alpha: float, beta: float) -> np.ndarray:
    """g(d) * DT^2 for d = 0..T-1 (float64)."""
    d = np.arange(T, dtype=np.float64)
    if abs(beta - alpha) > 1e-12:
        g = (beta ** (d + 1) - alpha ** (d + 1)) / (beta - alpha)
    else:
        g = (d + 1) * alpha**d
    return g * DT * DT


def _build_gt(alpha: float, beta: float) -> np.ndarray:
    """GT[s, t] = DT^2 * g(t - s) for s <= t else 0 (upper-triangular)."""
    g = _filter_taps(alpha, beta)
    s = np.arange(T)
    diff = s[None, :] - s[:, None]  # diff[s, t] = t - s
    gt = np.where(diff >= 0, g[np.clip(diff, 0, T - 1)], 0.0)
    return gt.astype(np.float32)


def _build_device():
    """Compile the per-core Tile kernel; returns the Bass module.

    Input layouts (pre-packed on the host; >=1KB per-partition DMA lines):
      gxa [P, 2, 2, 512]     fp8: [:, 0] = gt s-tile 0, [:, 1] = x_b0 s-tile 0
      gxb [P, 2, 2, 512]     fp8: [:, 0] = gt s-tile 1, [:, 1] = x_b0 s-tile 1
      xc  [P, 2, 2, 512]     fp8: x_b1, [:, ks] = s-tile ks
                             (rows: s = (2*ks+s2)*128 + p, scaled by sxx/sgt)
      w   [P, NCH, KP, 2, 512] fp8: w[p, c, kp, i2, n]
                             = W_eff[(2kp+i2)*128+p, c*512+n] * sw
      sc  [P, 2]             f32: +th', -th'  (th' = sxx*sgt*sw)
      spk [NB_LOC, KT, P, NW] fp8 == per batch [512 t, 2048 n] row-major
    """
    nc = bacc.Bacc(
        "TRN2", target_bir_lowering=False, debug=False, num_devices=NCORES
    )
    gxa = nc.dram_tensor("gxa", [P, 2, 2, 512], FP8, kind="ExternalInput").ap()
    gxb = nc.dram_tensor("gxb", [P, 2, 2, 512], FP8, kind="ExternalInput").ap()
    xc = nc.dram_tensor("xc", [P, 2, 2, 512], FP8, kind="ExternalInput").ap()
    w = nc.dram_tensor(
        "w", [P, NCH, KP, 2, 512], FP8, kind="ExternalInput"
    ).ap()
    sc = nc.dram_tensor("sc", [P, 2], F32, kind="ExternalInput").ap()
    spk = nc.dram_tensor(
        "spk", [NB_LOC, KT, P, NW], FP8, kind="ExternalOutput"
    ).ap()

    DR = mybir.MatmulPerfMode.DoubleRow
    H = T // 2

    with tile.TileContext(nc) as tc:
        with (
            tc.tile_pool(name="const", bufs=1) as cpool,
            tc.tile_pool(name="sout", bufs=4) as spool,
            tc.tile_pool(name="pp", bufs=4, space="PSUM") as pp,
        ):
            # --- input DMAs: everything bandwidth-critical on the two
            # HWDGE rings, in consumption order; tiny sc on SWDGE --------
            dummy = cpool.tile([P, 640], FP8, tag="dummy")
            nc.vector.memset(dummy, 0.0)
            gxa_sb = cpool.tile([P, 2, 2, 512], FP8, tag="gxa")
            nc.sync.dma_start(gxa_sb, gxa)
            gxb_sb = cpool.tile([P, 2, 2, 512], FP8, tag="gxb")
            nc.scalar.dma_start(gxb_sb, gxb)
            xc_sb = cpool.tile([P, 2, 2, 512], FP8, tag="xc")
            nc.scalar.dma_start(xc_sb, xc)
            sc_sb = cpool.tile([P, 2], F32, tag="sc")
            nc.gpsimd.dma_start(sc_sb, sc)
            w_sb = cpool.tile([P, NCH, KP, 2, 512], FP8, tag="w")
            for c in range(NCH):
                eng = nc.sync if c % 2 == 0 else nc.scalar
                eng.dma_start(w_sb[:, c], w[:, c])
            # force the Sign ACT table load now (overlapped with the input
            # DMA) instead of right before the first real compare
            warm_act = cpool.tile([P, 1], FP8, tag="wact")
            nc.scalar.activation(
                warm_act,
                dummy[:, 0:1],
                mybir.ActivationFunctionType.Sign,
                bias=0.0,
            )

            def gt_ap(ks, tlo, thi):  # gt rows of s-tile ks, t in [tlo,thi)
                src = gxa_sb if ks == 0 else gxb_sb
                return src[:, 0, :, tlo:thi]

            def x_ap(b, ks, m):  # x_b rows of s-tile ks, i-block m
                if b == 0:
                    src = (gxa_sb if ks == 0 else gxb_sb)[:, 1]
                else:
                    src = xc_sb[:, ks]
                return src[:, :, m * P : (m + 1) * P]

            xg_sb = [
                [
                    cpool.tile(
                        [P, 2, T], FP8, tag=f"xg{b}{kp}", name=f"xg{b}{kp}"
                    )
                    for kp in range(KP)
                ]
                for b in range(NB_LOC)
            ]

            # PE warm-up: junk matmuls with no DMA dependency keep the PE
            # HAM activity window busy from kernel start, so the 1.2 ->
            # 2.4 GHz un-throttle fires before the real matmuls.
            wp = pp.tile([P, 2, T], F32, tag="p", name="warm")
            for i in range(N_WARM):
                nc.tensor.matmul(
                    wp[:, 0, :],
                    dummy[:, 0:P],
                    dummy[:, P : P + 512],
                    start=True,
                    stop=True,
                    skip_group_check=True,
                )

            # --- stage 1: xgT[i, t] = sum_s x_b[s, i] * GT[s, t] --------
            # i-pair kp holds i-blocks m = 2kp (i2=0) and 2kp+1 (i2=1).
            # GT[s, t] == 0 for t < s, so xg[:, t < 256] only needs s-tile
            # 0: compute that "early" part first (own accumulation groups)
            # and cast it out immediately.
            p1 = {}
            for b in range(NB_LOC):
                for kp in range(KP):
                    p1[b, kp] = pp.tile(
                        [P, 2, T], F32, tag="p", name=f"p1_{b}{kp}"
                    )
            for b in range(NB_LOC):
                for kp in range(KP):
                    for i2 in range(2):
                        nc.tensor.matmul(
                            p1[b, kp][:, i2, 0:H],
                            x_ap(b, 0, 2 * kp + i2),
                            gt_ap(0, 0, H),
                            start=True,
                            stop=True,
                            perf_mode=DR,
                            skip_group_check=True,
                        )
                for kp in range(KP):  # early cast: xg[:, :, 0:256]
                    dst = xg_sb[b][kp][:, :, 0:H]
                    src = p1[b, kp][:, :, 0:H]
                    if (b + kp) % 2 == 0:
                        nc.scalar.copy(dst, src)
                    else:
                        nc.vector.tensor_copy(dst, src)
                for kp in range(KP):
                    for i2 in range(2):
                        for ks in range(KP):
                            nc.tensor.matmul(
                                p1[b, kp][:, i2, H:],
                                x_ap(b, ks, 2 * kp + i2),
                                gt_ap(ks, H, T),
                                start=(ks == 0),
                                stop=(ks == KP - 1),
                                perf_mode=DR,
                                skip_group_check=True,
                            )
                for kp in range(KP):  # late cast: xg[:, :, 256:]
                    dst = xg_sb[b][kp][:, :, H:]
                    src = p1[b, kp][:, :, H:]
                    if (b + kp) % 2 == 0:
                        nc.scalar.copy(dst, src)
                    else:
                        nc.vector.tensor_copy(dst, src)

            # --- stage 2: mem[t, n] = sum_i xgT[i, t] * W_eff[i, n] -----
            # per 1024-wide PSUM tile: kp0 into both halves then kp1,
            # sharing the stationary operand between consecutive matmuls.
            idx = 0
            for b in range(NB_LOC):
                for mt in range(KT):
                    s_sb = spool.tile(
                        [P, NW], FP8, tag="s", name=f"s{b}{mt}"
                    )
                    for u in range(2):
                        p2 = pp.tile(
                            [P, 1024], F32, tag="p", name=f"p2_{b}{mt}{u}"
                        )
                        for kp in range(KP):
                            for jh in range(2):
                                c = u * 2 + jh
                                nc.tensor.matmul(
                                    p2[:, jh * 512 : (jh + 1) * 512],
                                    xg_sb[b][kp][:, :, mt * P : (mt + 1) * P],
                                    w_sb[:, c, kp],
                                    start=(kp == 0),
                                    stop=(kp == KP - 1),
                                    perf_mode=DR,
                                    skip_group_check=True,
                                )
                        s_out = s_sb[:, u * 1024 : (u + 1) * 1024]
                        if idx % 2 == 0:
                            # sign(mem - th) in {-1, 0, 1}; host maps >0
                            # to spikes
                            nc.scalar.activation(
                                s_out,
                                p2,
                                mybir.ActivationFunctionType.Sign,
                                bias=sc_sb[:, 1:2],
                            )
                        else:
                            nc.vector.tensor_scalar(
                                s_out,
                                p2,
                                sc_sb[:, 0:1],
                                None,
                                op0=mybir.AluOpType.is_ge,
                            )
                        idx += 1
                    nc.sync.dma_start(spk[b, mt], s_sb)
    nc.compile()
    return nc


def _pow2_scale(target_max: float, value_max: float) -> float:
    """Largest power of two s with value_max * s <= target_max."""
    if value_max <= 0 or not np.isfinite(value_max):
        return 1.0
    return 2.0 ** math.floor(math.log2(target_max / value_max))


def _run_spmd_with_retry(nc, in_maps, trace=False, tries=3):
    """run_bass_kernel_spmd with retry: execution occasionally dies with a
    transient NRT error (device left wedged by a previous process).  A
    plain retry usually fails in-process, so later attempts reset the jax
    backend to get a fresh PJRT client."""
    import time as _time

    last = None
    for attempt in range(tries):
        try:
            return bass_utils.run_bass_kernel_spmd(
                nc, in_maps, core_ids=list(range(NCORES)), trace=trace
            )
        except Exception as e:  # noqa: BLE001
            last = e
            _time.sleep(2.0)
            try:
                import jax

                jax.clear_caches()
                jax.extend.backend.clear_backends()
            except Exception:  # noqa: BLE001
                pass
    raise last


def _run_device(x_all, W_eff, gt_np, sw, sxx, sgt, trace=False):
    """Run the SPMD kernel; returns (spikes [T,B,N] f32, results obj).

    x_all: [B, T, IN] f32 (per-batch time-major); W_eff: [IN, N] f32
    (threshold already folded in); scales are powers of two.
    """
    if True not in _compiled:
        _compiled[True] = _build_device()
    nc = _compiled[True]
    x_f8 = (x_all.astype(np.float64) * sxx).astype(np.float32).astype(NPFP8)
    gt_f8 = (gt_np.astype(np.float64) * sgt).astype(np.float32).astype(NPFP8)
    # gt[p, ks, s2, t] = GT[(2ks+s2)*128+p, t] * sgt
    gt_pack = gt_f8.reshape(KP, 2, P, T).transpose(2, 0, 1, 3)
    # x[p, ks, s2, i] = x_b[(2ks+s2)*128+p, i] * sxx  (per batch)
    x_pack = x_f8.reshape(B, KP, 2, P, IN).transpose(0, 3, 1, 2, 4)
    w_f8 = (W_eff.astype(np.float64) * sw).astype(np.float32).astype(NPFP8)
    # w[p, c, kp, i2, n] = W_eff[(2kp+i2)*128+p, c*512+n] * sw; per ng half
    w_pack = [
        np.ascontiguousarray(
            w_f8[:, ng * NW : (ng + 1) * NW]
            .reshape(KP, 2, P, NCH, 512)
            .transpose(2, 3, 0, 1, 4)
        )
        for ng in range(NNG)
    ]
    thp = float(sxx * sgt * sw)
    sc_arr = np.empty((P, 2), dtype=np.float32)
    sc_arr[:, 0] = thp
    sc_arr[:, 1] = -thp
    in_maps = []
    for core in range(NCORES):
        bg, ng = divmod(core, NNG)
        b0, b1 = bg * NB_LOC, bg * NB_LOC + 1
        in_maps.append(
            {
                "gxa": np.ascontiguousarray(
                    np.stack([gt_pack[:, 0], x_pack[b0][:, 0]], axis=1)
                ),
                "gxb": np.ascontiguousarray(
                    np.stack([gt_pack[:, 1], x_pack[b0][:, 1]], axis=1)
                ),
                "xc": np.ascontiguousarray(x_pack[b1]),
                "w": w_pack[ng],
                "sc": sc_arr,
            }
        )
    res = _run_spmd_with_retry(nc, in_maps, trace=trace)
    global LAST_RES
    LAST_RES = res
    out = np.empty((T, B, N), dtype=np.float32)
    for core in range(NCORES):
        bg, ng = divmod(core, NNG)
        s = res.results[core]["spk"].astype(np.float32)  # [NB_LOC,KT,P,NW]
        s = (s > 0).astype(np.float32).reshape(NB_LOC, T, NW)
        for b in range(NB_LOC):
            out[:, bg * NB_LOC + b, ng * NW : (ng + 1) * NW] = s[b]
    return out, res


def _fallback(input_signal, weights, tau_mem, tau_syn, threshold):
    """Exact sequential port of the reference (numpy float32)."""
    x = np.asarray(input_signal, dtype=np.float32)
    w = np.asarray(weights, dtype=np.float32)
    W_in, W_rec = w[:IN], w[IN:]
    Tt, Bb, Nn = x.shape
    ff = np.einsum("tbi,in->tbn", x[:, :, :IN], W_in).astype(np.float32)
    syn = np.zeros((Bb, Nn), np.float32)
    mem = np.zeros((Bb, Nn), np.float32)
    fb = np.zeros((Bb, Nn), np.float32)
    out = np.zeros((Tt, Bb, Nn), np.float32)
    for t in range(Tt):
        cur = ff[t] + fb
        syn = syn + (-syn / tau_syn + cur) * np.float32(DT)
        mem = mem + (-mem / tau_mem + syn) * np.float32(DT)
        spikes = (mem >= threshold).astype(np.float32)
        mem = mem * (1.0 - spikes)
        rec = spikes[:, IN:] @ W_rec
        rec[:, :IN] = 0.0
        fb = rec
        out[t] = spikes
    return out


def kernel(input_signal, weights, tau_mem, tau_syn, threshold, _trace=False):
    input_signal = np.asarray(input_signal)
    weights = np.asarray(weights)
    tau_mem = np.asarray(tau_mem)
    tau_syn = np.asarray(tau_syn)
    threshold = np.asarray(threshold)

    ok_shape = (
        input_signal.shape == (T, B, N)
        and weights.shape == (N, N)
        and np.all(tau_mem == tau_mem.flat[0])
        and np.all(tau_syn == tau_syn.flat[0])
        and np.all(np.isfinite(input_signal))
        and np.all(np.isfinite(weights[:IN]))
        and np.all(np.isfinite(threshold))
        and np.all(threshold > 0)
    )
    if not ok_shape:
        return _fallback(input_signal, weights, tau_mem, tau_syn, threshold)

    alpha = 1.0 - DT / float(tau_syn.flat[0])
    beta = 1.0 - DT / float(tau_mem.flat[0])
    if not (0.0 <= alpha < 1.0 and 0.0 <= beta < 1.0):
        # numerically unstable / nonstandard regime: be safe
        return _fallback(input_signal, weights, tau_mem, tau_syn, threshold)

    gt_np = _build_gt(alpha, beta)

    # --- rigorous sub-threshold bound (exact arithmetic, fp64) -----------
    # Fold threshold into W:  spikes = (mem/th >= 1), W_eff = W_in / th.
    x_in = input_signal[:, :, :IN].astype(np.float64)
    th64 = threshold.astype(np.float64)
    W_eff64 = weights[:IN].astype(np.float64) / th64[None, :]
    if not np.all(np.isfinite(W_eff64)):
        return _fallback(input_signal, weights, tau_mem, tau_syn, threshold)

    # 2-norm machinery:
    #   |mem'[t,n]| <= ||xg[:,t]||_2 * ||W_eff[:,n]||_2
    #   ||xg[:,t]||_2 <= sum_d g(d)DT^2 * max_t||x[t,:]||_2  (triangle ineq)
    #   |xg[i,t]|     <= max_i||x[:,i]||_2 * max_t||gt[:,t]||_2
    max_row = float(np.sqrt((x_in * x_in).sum(axis=2).max()))
    max_wcol = float(np.sqrt((W_eff64 * W_eff64).sum(axis=0).max()))
    gsum = float(_filter_taps(alpha, beta).sum())
    xg_col2 = gsum * max_row               # bound on ||xg[:,t]||_2
    mem_bound = xg_col2 * max_wcol         # bound on true |mem'|
    xcol_max = float(np.sqrt((x_in * x_in).sum(axis=0).max()))
    gt64 = gt_np.astype(np.float64)
    gtcol_max = float(np.sqrt((gt64 * gt64).sum(axis=0).max()))
    xg_bound = xcol_max * gtcol_max        # bound on |xg[i,t]|
    w_max = float(np.abs(W_eff64).max())
    x_max = float(np.abs(x_in).max())

    # fp8 power-of-two scales.  sgt := sx/sxx makes the stage-1 PSUM value
    # exactly xg * (sxx*sgt) <= 224, so the PSUM->SBUF copy is a pure cast.
    sxx = _pow2_scale(224.0, x_max)
    sx = _pow2_scale(224.0, xg_bound)
    sgt = sx / sxx
    sw = _pow2_scale(224.0, w_max)
    if not (np.isfinite(sgt) and sgt > 0):
        return _fallback(input_signal, weights, tau_mem, tau_syn, threshold)
    gt_fp8_max = float(np.abs(gt64).max()) * sgt
    if gt_fp8_max > 448.0:  # would overflow fp8-e4m3
        return _fallback(input_signal, weights, tau_mem, tau_syn, threshold)

    # --- mixed-precision error budget (conservative, absolute) ----------
    # fp8-e4m3 rounding: rel 2^-4 plus subnormal-flush floor 2^-9/scale;
    # products accumulate in fp32 PSUM (rel ~2^-20 slop folded in at the
    # end).  Per element:
    #   stage-1 product error  e1 <= 0.13*xg_bound
    #        + (2^-9/sgt)*sqrt(T)*xcol_max + (2^-9/sxx)*sqrt(T)*gtcol_max
    #        + T*2^-18/(sxx*sgt)
    #   xg cast adds rel 2^-4 + flush:  XGE = 1.0625*e1 + 0.0625*xg_bound
    #        + 2^-9/(sxx*sgt)
    #   stage-2:  |p2' - mem'| <= sqrt(IN)*max_wcol*XGE + 0.0625*mem_bound
    #        + (2^-9/sw)*sqrt(IN)*xg_col2 + IN*XGE*(0.0625*w_max+2^-9/sw)
    sqT = math.sqrt(T)
    sqI = math.sqrt(IN)
    fl_gt = 2.0**-9 / sgt
    fl_xx = 2.0**-9 / sxx
    fl_xg = 2.0**-9 / (sxx * sgt)
    fl_w = 2.0**-9 / sw
    e1 = (
        0.13 * xg_bound
        + fl_gt * sqT * xcol_max
        + fl_xx * sqT * gtcol_max
        + T * 2.0**-18 / (sxx * sgt)
    )
    xge = 1.0625 * e1 + 0.0625 * xg_bound + fl_xg
    err = (
        sqI * max_wcol * xge
        + 0.0625 * mem_bound
        + fl_w * sqI * xg_col2
        + IN * xge * (0.0625 * w_max + fl_w)
    )
    total = (mem_bound + err) * 1.001  # fp32 accumulation slop
    if not (total < 1.0 - MARGIN):
        return _fallback(input_signal, weights, tau_mem, tau_syn, threshold)

    # [B, T, IN] per-batch time-major rows
    x_all = np.ascontiguousarray(
        input_signal[:, :, :IN].transpose(1, 0, 2)
    ).astype(np.float32, copy=False)
    W_eff = W_eff64.astype(np.float32)

    try:
        spikes, _ = _run_device(
            x_all, W_eff, gt_np, sw, sxx, sgt, trace=_trace
        )
    except Exception:  # device unusable: still return a correct result
        return _fallback(input_signal, weights, tau_mem, tau_syn, threshold)
    if spikes.any():
        # bound said sub-threshold yet device saw spikes: distrust, recompute
        return _fallback(input_signal, weights, tau_mem, tau_syn, threshold)
    return spikes


# revision 17
# speedup vs baseline: 1.0805x; 1.0805x over previous
"""Trainium2 Bass kernel for nn_EvolvableSNN (T=512, B=8, N=4096, LIF SNN).

Strategy
--------
The LIF dynamics with these parameters are sub-threshold: the membrane
potential equilibrium is ~tau_mem*tau_syn*cur ~= 1e-4 * cur, four orders of
magnitude below threshold=1.0, so no neuron ever spikes and the recurrent
feedback term is identically zero.  With zero feedback the scan is a LINEAR
time-invariant filter of the feedforward drive:

    ff    = input[:, :, :512] @ W_in                      # [T, B, N]
    mem_t = DT^2 * sum_{s<=t} g(t-s) * ff_s               # per (b, n)
    g(d)  = (b^(d+1) - a^(d+1)) / (b - a),  a = 1-DT/tau_syn, b = 1-DT/tau_mem
    spikes_t = (mem_t >= threshold)

so mem = GT.T @_time (x @ W_in) -- two chained dense matmuls, fully parallel
across (batch, neuron).  Validity is guarded by a rigorous norm bound
computed on the host (see kernel()); if the bound does not clear the
threshold by a wide margin -- or the device reports any spike -- we fall
back to an exact sequential numpy port of the reference.  The first spike
of the no-feedback system coincides with the first spike of the true
system, so "no spikes under linearization" exactly implies correctness.

Numerics: both matmul stages run as fp8-e4m3 DoubleRow (2x PE throughput)
with power-of-two scales; fp32 PSUM accumulation.  The threshold is folded
into W on the host (W_eff = W_in / th per column), so the device compare is
always against the single runtime scalar sxx*sgt*sw.  sgt is chosen as
sx/sxx so the stage-1 PSUM->SBUF copy is a PURE CAST (no scale multiply).

Sharding: 4 batch-groups x 2 neuron-column-groups over 8 cores (2 batches
and 2048 columns per core).  The column split halves the per-core weight
DMA (1 MB) -- per-queue DMA bandwidth, not PE time, is what gates an early
stage-2 start.  No collectives.

Schedule lessons baked in (from perfetto traces):
 * The PE HAM clock gate releases (1.2 -> 2.4 GHz) only after ~3.4us of
   GAP-FREE PE activity, and micro-stalls reset the window -- so junk
   warm-up matmuls run during the input DMA and every stage-2 dependency
   (w chunks, xg casts) must be strictly ahead of the PE.
 * Closing a scoped PSUM pool mid-kernel inserts an all-engine barrier --
   one shared pool holds stage-1 and stage-2 tiles.
 * SWDGE (gpsimd) DMA has ~2us fixed cost per transfer; everything
   bandwidth-critical goes on the two HWDGE rings (sync / scalar).
"""

import math

import numpy as np
import ml_dtypes

import concourse.bass as bass
import concourse.mybir as mybir
import concourse.tile as tile
from concourse import bacc, bass_utils

# Problem constants (hardcoded per harness contract).
T, B, N = 512, 8, 4096
IN = 512          # INPUT_SIZE
DT = 0.001
P = 128           # SBUF partitions
NCORES = 8

NBG, NNG = 4, 2   # core grid: batch-groups x neuron-column-groups
NB_LOC = B // NBG          # batches per core (2)
NW = N // NNG              # neuron columns per core (2048)
NCH = NW // 512            # 512-wide w chunks per core (4)
KI = IN // P               # contraction tiles over input dim (4)
KP = KI // 2               # DoubleRow contraction pair-tiles (2)
KT = T // P                # tiles over time dim (4)
N_WARM = 6                 # PE warm-up matmuls during the input DMA
# stage-2 w consumption order as (c, kp) pairs; w is DMA'd in this order
WORDER = [(0, 0), (1, 0), (0, 1), (1, 1), (2, 0), (3, 0), (2, 1), (3, 1)]
WIDX = {ck: j for j, ck in enumerate(WORDER)}
F32 = mybir.dt.float32
FP8 = mybir.dt.float8e4
NPFP8 = ml_dtypes.float8_e4m3

MARGIN = 0.1      # abs margin to threshold 1.0 for the fast path

_compiled = {}    # cached compiled Bass module
LAST_RES = None   # last device results (for external profiling)


def _filter_taps(alpha: float, beta: float) -> np.ndarray:
    """g(d) * DT^2 for d = 0..T-1 (float64)."""
    d = np.arange(T, dtype=np.float64)
    if abs(beta - alpha) > 1e-12:
        g = (beta ** (d + 1) - alpha ** (d + 1)) / (beta - alpha)
    else:
        g = (d + 1) * alpha**d
    return g * DT * DT


def _build_gt(alpha: float, beta: float) -> np.ndarray:
    """GT[s, t] = DT^2 * g(t - s) for s <= t else 0 (upper-triangular)."""
    g = _filter_taps(alpha, beta)
    s = np.arange(T)
    diff = s[None, :] - s[:, None]  # diff[s, t] = t - s
    gt = np.where(diff >= 0, g[np.clip(diff, 0, T - 1)], 0.0)
    return gt.astype(np.float32)


def _build_device():
    """Compile the per-core Tile kernel; returns the Bass module.

    Input layouts (pre-packed on the host; >=1KB per-partition DMA lines):
      gxa [P, 2, 2, 512]     fp8: [:, 0] = gt s-tile 0, [:, 1] = x_b0 s-tile 0
      gxb [P, 2, 2, 512]     fp8: [:, 0] = gt s-tile 1, [:, 1] = x_b0 s-tile 1
      xc  [P, 2, 2, 512]     fp8: x_b1, [:, ks] = s-tile ks
                             (rows: s = (2*ks+s2)*128 + p, scaled by sxx/sgt)
      w   [P, NCH, KP, 2, 512] fp8: w[p, c, kp, i2, n]
                             = W_eff[(2kp+i2)*128+p, c*512+n] * sw
      sc  [P, 2]             f32: +th', -th'  (th' = sxx*sgt*sw)
      spk [NB_LOC, KT, P, NW] fp8 == per batch [512 t, 2048 n] row-major
    """
    nc = bacc.Bacc(
        "TRN2", target_bir_lowering=False, debug=False, num_devices=NCORES
    )
    gxa = nc.dram_tensor("gxa", [P, 2, 2, 512], FP8, kind="ExternalInput").ap()
    gxb = nc.dram_tensor("gxb", [P, 2, 2, 512], FP8, kind="ExternalInput").ap()
    xc = nc.dram_tensor("xc", [P, 2, 2, 512], FP8, kind="ExternalInput").ap()
    # w blobs in exact stage-2 consumption order (c, kp); see WORDER
    w = nc.dram_tensor(
        "w", [P, NCH * KP, 2, 512], FP8, kind="ExternalInput"
    ).ap()
    sc = nc.dram_tensor("sc", [P, 2], F32, kind="ExternalInput").ap()
    spk = nc.dram_tensor(
        "spk", [NB_LOC, KT, P, NW], FP8, kind="ExternalOutput"
    ).ap()

    DR = mybir.MatmulPerfMode.DoubleRow
    H = T // 2

    with tile.TileContext(nc) as tc:
        with (
            tc.tile_pool(name="const", bufs=1) as cpool,
            tc.tile_pool(name="sout", bufs=4) as spool,
            tc.tile_pool(name="pp", bufs=4, space="PSUM") as pp,
        ):
            # --- input DMAs: everything bandwidth-critical on the two
            # HWDGE rings, in consumption order; tiny sc on SWDGE --------
            dummy = cpool.tile([P, 640], FP8, tag="dummy")
            nc.vector.memset(dummy, 0.0)
            gxa_sb = cpool.tile([P, 2, 2, 512], FP8, tag="gxa")
            nc.sync.dma_start(gxa_sb, gxa)
            xc_sb = cpool.tile([P, 2, 2, 512], FP8, tag="xc")
            nc.scalar.dma_start(xc_sb, xc)
            gxb_sb = cpool.tile([P, 2, 2, 512], FP8, tag="gxb")
            nc.sync.dma_start(gxb_sb, gxb)
            sc_sb = cpool.tile([P, 2], F32, tag="sc")
            nc.gpsimd.dma_start(sc_sb, sc)
            w_sb = cpool.tile([P, NCH * KP, 2, 512], FP8, tag="w")
            for j in range(NCH * KP):  # 128KB apiece, consumption order
                eng = nc.sync if j % 2 == 0 else nc.scalar
                eng.dma_start(w_sb[:, j], w[:, j])
            # force the Sign ACT table load now (overlapped with the input
            # DMA) instead of right before the first real compare
            warm_act = cpool.tile([P, 1], FP8, tag="wact")
            nc.scalar.activation(
                warm_act,
                dummy[:, 0:1],
                mybir.ActivationFunctionType.Sign,
                bias=0.0,
            )

            def gt_ap(ks, tlo, thi):  # gt rows of s-tile ks, t in [tlo,thi)
                src = gxa_sb if ks == 0 else gxb_sb
                return src[:, 0, :, tlo:thi]

            def x_ap(b, ks, m):  # x_b rows of s-tile ks, i-block m
                if b == 0:
                    src = (gxa_sb if ks == 0 else gxb_sb)[:, 1]
                else:
                    src = xc_sb[:, ks]
                return src[:, :, m * P : (m + 1) * P]

            xg_sb = [
                [
                    cpool.tile(
                        [P, 2, T], FP8, tag=f"xg{b}{kp}", name=f"xg{b}{kp}"
                    )
                    for kp in range(KP)
                ]
                for b in range(NB_LOC)
            ]

            # PE warm-up: junk matmuls with no DMA dependency keep the PE
            # HAM activity window busy from kernel start, so the 1.2 ->
            # 2.4 GHz un-throttle fires before the real matmuls.
            wp = pp.tile([P, 2, T], F32, tag="p", name="warm")
            for i in range(N_WARM):
                nc.tensor.matmul(
                    wp[:, 0, :],
                    dummy[:, 0:P],
                    dummy[:, P : P + 512],
                    start=True,
                    stop=True,
                    skip_group_check=True,
                )

            # --- stage 1: xgT[i, t] = sum_s x_b[s, i] * GT[s, t] --------
            # i-pair kp holds i-blocks m = 2kp (i2=0) and 2kp+1 (i2=1).
            # GT[s, t] == 0 for t < s: s-tile ks only feeds t >= 256*ks.
            # Big moving operands (F=512/256) keep the per-matmul
            # LDWEIGHTS (~229ns for DoubleRow) hidden under streaming.
            p1 = {}
            for b in range(NB_LOC):
                for kp in range(KP):
                    p1[b, kp] = pp.tile(
                        [P, 2, T], F32, tag="p", name=f"p1_{b}{kp}"
                    )
            for b in range(NB_LOC):
                for kp in range(KP):
                    for i2 in range(2):
                        for ks in range(KP):
                            t0 = ks * H
                            nc.tensor.matmul(
                                p1[b, kp][:, i2, t0:],
                                x_ap(b, ks, 2 * kp + i2),
                                gt_ap(ks, t0, T),
                                start=(ks == 0),
                                stop=(ks == KP - 1),
                                perf_mode=DR,
                                skip_group_check=True,
                            )
                for kp in range(KP):  # cast whole xg_b[kp] (FD 1024)
                    if (b + kp) % 2 == 0:
                        nc.scalar.copy(xg_sb[b][kp], p1[b, kp])
                    else:
                        nc.vector.tensor_copy(xg_sb[b][kp], p1[b, kp])

            # --- stage 2: mem[t, n] = sum_i xgT[i, t] * W_eff[i, n] -----
            # per 1024-wide PSUM tile: kp0 into both halves then kp1,
            # sharing the stationary operand between consecutive matmuls.
            idx = 0
            for b in range(NB_LOC):
                for mt in range(KT):
                    s_sb = spool.tile(
                        [P, NW], FP8, tag="s", name=f"s{b}{mt}"
                    )
                    for u in range(2):
                        p2 = pp.tile(
                            [P, 1024], F32, tag="p", name=f"p2_{b}{mt}{u}"
                        )
                        for kp in range(KP):
                            for jh in range(2):
                                c = u * 2 + jh
                                nc.tensor.matmul(
                                    p2[:, jh * 512 : (jh + 1) * 512],
                                    xg_sb[b][kp][:, :, mt * P : (mt + 1) * P],
                                    w_sb[:, WIDX[c, kp]],
                                    start=(kp == 0),
                                    stop=(kp == KP - 1),
                                    perf_mode=DR,
                                    skip_group_check=True,
                                )
                        s_out = s_sb[:, u * 1024 : (u + 1) * 1024]
                        if idx % 2 == 0:
                            # sign(mem - th) in {-1, 0, 1}; host maps >0
                            # to spikes
                            nc.scalar.activation(
                                s_out,
                                p2,
                                mybir.ActivationFunctionType.Sign,
                                bias=sc_sb[:, 1:2],
                            )
                        else:
                            nc.vector.tensor_scalar(
                                s_out,
                                p2,
                                sc_sb[:, 0:1],
                                None,
                                op0=mybir.AluOpType.is_ge,
                            )
                        idx += 1
                    nc.sync.dma_start(spk[b, mt], s_sb)
    nc.compile()
    return nc


def _pow2_scale(target_max: float, value_max: float) -> float:
    """Largest power of two s with value_max * s <= target_max."""
    if value_max <= 0 or not np.isfinite(value_max):
        return 1.0
    return 2.0 ** math.floor(math.log2(target_max / value_max))


def _run_spmd_with_retry(nc, in_maps, trace=False, tries=3):
    """run_bass_kernel_spmd with retry: execution occasionally dies with a
    transient NRT error (device left wedged by a previous process).  A
    plain retry usually fails in-process, so later attempts reset the jax
    backend to get a fresh PJRT client."""
    import time as _time

    last = None
    for attempt in range(tries):
        try:
            return bass_utils.run_bass_kernel_spmd(
                nc, in_maps, core_ids=list(range(NCORES)), trace=trace
            )
        except Exception as e:  # noqa: BLE001
            last = e
            _time.sleep(2.0)
            try:
                import jax

                jax.clear_caches()
                jax.extend.backend.clear_backends()
            except Exception:  # noqa: BLE001
                pass
    raise last


def _run_device(x_all, W_eff, gt_np, sw, sxx, sgt, trace=False):
    """Run the SPMD kernel; returns (spikes [T,B,N] f32, results obj).

    x_all: [B, T, IN] f32 (per-batch time-major); W_eff: [IN, N] f32
    (threshold already folded in); scales are powers of two.
    """
    if True not in _compiled:
        _compiled[True] = _build_device()
    nc = _compiled[True]
    x_f8 = (x_all.astype(np.float64) * sxx).astype(np.float32).astype(NPFP8)
    gt_f8 = (gt_np.astype(np.float64) * sgt).astype(np.float32).astype(NPFP8)
    # gt[p, ks, s2, t] = GT[(2ks+s2)*128+p, t] * sgt
    gt_pack = gt_f8.reshape(KP, 2, P, T).transpose(2, 0, 1, 3)
    # x[p, ks, s2, i] = x_b[(2ks+s2)*128+p, i] * sxx  (per batch)
    x_pack = x_f8.reshape(B, KP, 2, P, IN).transpose(0, 3, 1, 2, 4)
    w_f8 = (W_eff.astype(np.float64) * sw).astype(np.float32).astype(NPFP8)
    # w[p, j, i2, n] = W_eff[(2kp+i2)*128+p, c*512+n] * sw with
    # (c, kp) = WORDER[j]; per ng column-half
    w_pack = []
    for ng in range(NNG):
        wc = (
            w_f8[:, ng * NW : (ng + 1) * NW]
            .reshape(KP, 2, P, NCH, 512)
            .transpose(2, 3, 0, 1, 4)  # [P, c, kp, i2, n]
        )
        w_pack.append(
            np.ascontiguousarray(
                np.stack([wc[:, c, kp] for c, kp in WORDER], axis=1)
            )
        )
    thp = float(sxx * sgt * sw)
    sc_arr = np.empty((P, 2), dtype=np.float32)
    sc_arr[:, 0] = thp
    sc_arr[:, 1] = -thp
    in_maps = []
    for core in range(NCORES):
        bg, ng = divmod(core, NNG)
        b0, b1 = bg * NB_LOC, bg * NB_LOC + 1
        in_maps.append(
            {
                "gxa": np.ascontiguousarray(
                    np.stack([gt_pack[:, 0], x_pack[b0][:, 0]], axis=1)
                ),
                "gxb": np.ascontiguousarray(
                    np.stack([gt_pack[:, 1], x_pack[b0][:, 1]], axis=1)
                ),
                "xc": np.ascontiguousarray(x_pack[b1]),
                "w": w_pack[ng],
                "sc": sc_arr,
            }
        )
    res = _run_spmd_with_retry(nc, in_maps, trace=trace)
    global LAST_RES
    LAST_RES = res
    out = np.empty((T, B, N), dtype=np.float32)
    for core in range(NCORES):
        bg, ng = divmod(core, NNG)
        s = res.results[core]["spk"].astype(np.float32)  # [NB_LOC,KT,P,NW]
        s = (s > 0).astype(np.float32).reshape(NB_LOC, T, NW)
        for b in range(NB_LOC):
            out[:, bg * NB_LOC + b, ng * NW : (ng + 1) * NW] = s[b]
    return out, res


def _fallback(input_signal, weights, tau_mem, tau_syn, threshold):
    """Exact sequential port of the reference (numpy float32)."""
    x = np.asarray(input_signal, dtype=np.float32)
    w = np.asarray(weights, dtype=np.float32)
    W_in, W_rec = w[:IN], w[IN:]
    Tt, Bb, Nn = x.shape
    ff = np.einsum("tbi,in->tbn", x[:, :, :IN], W_in).astype(np.float32)
    syn = np.zeros((Bb, Nn), np.float32)
    mem = np.zeros((Bb, Nn), np.float32)
    fb = np.zeros((Bb, Nn), np.float32)
    out = np.zeros((Tt, Bb, Nn), np.float32)
    for t in range(Tt):
        cur = ff[t] + fb
        syn = syn + (-syn / tau_syn + cur) * np.float32(DT)
        mem = mem + (-mem / tau_mem + syn) * np.float32(DT)
        spikes = (mem >= threshold).astype(np.float32)
        mem = mem * (1.0 - spikes)
        rec = spikes[:, IN:] @ W_rec
        rec[:, :IN] = 0.0
        fb = rec
        out[t] = spikes
    return out


def kernel(input_signal, weights, tau_mem, tau_syn, threshold, _trace=False):
    input_signal = np.asarray(input_signal)
    weights = np.asarray(weights)
    tau_mem = np.asarray(tau_mem)
    tau_syn = np.asarray(tau_syn)
    threshold = np.asarray(threshold)

    ok_shape = (
        input_signal.shape == (T, B, N)
        and weights.shape == (N, N)
        and np.all(tau_mem == tau_mem.flat[0])
        and np.all(tau_syn == tau_syn.flat[0])
        and np.all(np.isfinite(input_signal))
        and np.all(np.isfinite(weights[:IN]))
        and np.all(np.isfinite(threshold))
        and np.all(threshold > 0)
    )
    if not ok_shape:
        return _fallback(input_signal, weights, tau_mem, tau_syn, threshold)

    alpha = 1.0 - DT / float(tau_syn.flat[0])
    beta = 1.0 - DT / float(tau_mem.flat[0])
    if not (0.0 <= alpha < 1.0 and 0.0 <= beta < 1.0):
        # numerically unstable / nonstandard regime: be safe
        return _fallback(input_signal, weights, tau_mem, tau_syn, threshold)

    gt_np = _build_gt(alpha, beta)

    # --- rigorous sub-threshold bound (exact arithmetic, fp64) -----------
    # Fold threshold into W:  spikes = (mem/th >= 1), W_eff = W_in / th.
    x_in = input_signal[:, :, :IN].astype(np.float64)
    th64 = threshold.astype(np.float64)
    W_eff64 = weights[:IN].astype(np.float64) / th64[None, :]
    if not np.all(np.isfinite(W_eff64)):
        return _fallback(input_signal, weights, tau_mem, tau_syn, threshold)

    # 2-norm machinery:
    #   |mem'[t,n]| <= ||xg[:,t]||_2 * ||W_eff[:,n]||_2
    #   ||xg[:,t]||_2 <= sum_d g(d)DT^2 * max_t||x[t,:]||_2  (triangle ineq)
    #   |xg[i,t]|     <= max_i||x[:,i]||_2 * max_t||gt[:,t]||_2
    max_row = float(np.sqrt((x_in * x_in).sum(axis=2).max()))
    max_wcol = float(np.sqrt((W_eff64 * W_eff64).sum(axis=0).max()))
    gsum = float(_filter_taps(alpha, beta).sum())
    xg_col2 = gsum * max_row               # bound on ||xg[:,t]||_2
    mem_bound = xg_col2 * max_wcol         # bound on true |mem'|
    xcol_max = float(np.sqrt((x_in * x_in).sum(axis=0).max()))
    gt64 = gt_np.astype(np.float64)
    gtcol_max = float(np.sqrt((gt64 * gt64).sum(axis=0).max()))
    xg_bound = xcol_max * gtcol_max        # bound on |xg[i,t]|
    w_max = float(np.abs(W_eff64).max())
    x_max = float(np.abs(x_in).max())

    # fp8 power-of-two scales.  sgt := sx/sxx makes the stage-1 PSUM value
    # exactly xg * (sxx*sgt) <= 224, so the PSUM->SBUF copy is a pure cast.
    sxx = _pow2_scale(224.0, x_max)
    sx = _pow2_scale(224.0, xg_bound)
    sgt = sx / sxx
    sw = _pow2_scale(224.0, w_max)
    if not (np.isfinite(sgt) and sgt > 0):
        return _fallback(input_signal, weights, tau_mem, tau_syn, threshold)
    gt_fp8_max = float(np.abs(gt64).max()) * sgt
    if gt_fp8_max > 448.0:  # would overflow fp8-e4m3
        return _fallback(input_signal, weights, tau_mem, tau_syn, threshold)

    # --- mixed-precision error budget (conservative, absolute) ----------
    # fp8-e4m3 rounding: rel 2^-4 plus subnormal-flush floor 2^-9/scale;
    # products accumulate in fp32 PSUM (rel ~2^-20 slop folded in at the
    # end).  Per element:
    #   stage-1 product error  e1 <= 0.13*xg_bound
    #        + (2^-9/sgt)*sqrt(T)*xcol_max + (2^-9/sxx)*sqrt(T)*gtcol_max
    #        + T*2^-18/(sxx*sgt)
    #   xg cast adds rel 2^-4 + flush:  XGE = 1.0625*e1 + 0.0625*xg_bound
    #        + 2^-9/(sxx*sgt)
    #   stage-2:  |p2' - mem'| <= sqrt(IN)*max_wcol*XGE + 0.0625*mem_bound
    #        + (2^-9/sw)*sqrt(IN)*xg_col2 + IN*XGE*(0.0625*w_max+2^-9/sw)
    sqT = math.sqrt(T)
    sqI = math.sqrt(IN)
    fl_gt = 2.0**-9 / sgt
    fl_xx = 2.0**-9 / sxx
    fl_xg = 2.0**-9 / (sxx * sgt)
    fl_w = 2.0**-9 / sw
    e1 = (
        0.13 * xg_bound
        + fl_gt * sqT * xcol_max
        + fl_xx * sqT * gtcol_max
        + T * 2.0**-18 / (sxx * sgt)
    )
    xge = 1.0625 * e1 + 0.0625 * xg_bound + fl_xg
    err = (
        sqI * max_wcol * xge
        + 0.0625 * mem_bound
        + fl_w * sqI * xg_col2
        + IN * xge * (0.0625 * w_max + fl_w)
    )
    total = (mem_bound + err) * 1.001  # fp32 accumulation slop
    if not (total < 1.0 - MARGIN):
        return _fallback(input_signal, weights, tau_mem, tau_syn, threshold)

    # [B, T, IN] per-batch time-major rows
    x_all = np.ascontiguousarray(
        input_signal[:, :, :IN].transpose(1, 0, 2)
    ).astype(np.float32, copy=False)
    W_eff = W_eff64.astype(np.float32)

    try:
        spikes, _ = _run_device(
            x_all, W_eff, gt_np, sw, sxx, sgt, trace=_trace
        )
    except Exception:  # device unusable: still return a correct result
        return _fallback(input_signal, weights, tau_mem, tau_syn, threshold)
    if spikes.any():
        # bound said sub-threshold yet device saw spikes: distrust, recompute
        return _fallback(input_signal, weights, tau_mem, tau_syn, threshold)
    return spikes
